# revision 1
# baseline (speedup 1.0000x reference)
"""Item2Vec negative-sampling loss on 8 Trainium2 NeuronCores.

Strategy (data-parallel over batch, tables replicated per core):
  - Each core handles B/8 = 2048 batch elements.
  - Embedding tables are converted to bf16 on host (final scalar rel err vs
    the f32 reference ~5e-5, verified numerically) and replicated to each
    core's HBM.
  - On-device: big indirect-DMA gathers (thousands of rows per instruction,
    amortizing the ~1us SWDGE fixed cost), then per 128-batch tile:
        prod  = neg/pos_embs * broadcast(center_emb)   (DVE, in-place)
        score = reduce_X(prod)                          (DVE, [128, 21])
    then one fused activation chain for the whole core:
        sig = Sigmoid(+/- score); l = Ln(sig + 1e-10)   (ACT)
        partial = sum(l)                                (DVE reduce + PE ones-matmul)
  - Host sums the 8 per-core partials: loss = -sum/B.
"""

import numpy as np
import ml_dtypes
from contextlib import ExitStack

import concourse.bass as bass
import concourse.bacc as bacc
import concourse.mybir as mybir
import concourse.tile as tile
from concourse.bass_utils import run_bass_kernel_spmd

V = 1_000_000
D = 128
B = 16384
NEG = 20
P = 128
NCORES = 8
BPC = B // NCORES           # 2048 batch elements per core
NTILES = BPC // P           # 16 batch tiles per core
K1 = NEG + 1                # pos + negs per batch element

PN_GRP = 2                  # batch tiles per pos/neg gather instruction
N_PN = NTILES // PN_GRP     # 8 gather instructions from W_out
C_GRP = 8                   # batch tiles per center gather instruction
N_C = NTILES // C_GRP       # 2 gather instructions from W_in

DT_TAB = mybir.dt.bfloat16
NP_TAB = ml_dtypes.bfloat16

TRACE = False
LAST_RESULTS = None
DEBUG_DUMP = False   # adds scores/lg debug outputs to the built kernel

_NC = None


def _body(ctx, tc, w_in, w_out, cidx, pnidx, out, dbg=None):
    nc = tc.nc
    f32 = mybir.dt.float32
    X = mybir.AxisListType.X
    AF = mybir.ActivationFunctionType

    idx_pool = ctx.enter_context(tc.tile_pool(name="idx", bufs=1))
    pn_pool = ctx.enter_context(tc.tile_pool(name="pn", bufs=3))
    c_pool = ctx.enter_context(tc.tile_pool(name="c", bufs=1))
    sc_pool = ctx.enter_context(tc.tile_pool(name="sc", bufs=1))
    ps_pool = ctx.enter_context(tc.tile_pool(name="ps", bufs=1, space="PSUM"))

    # NOTE: each gather instruction gets its own CONTIGUOUS offset tile —
    # the HW SWDGE mishandles strided (column-sliced) offset APs even
    # though CoreSim models them fine.
    c_offs = []
    for g in range(N_C):
        ot = idx_pool.tile([P, C_GRP], mybir.dt.int32, tag=f"coff{g}")
        nc.sync.dma_start(out=ot[:], in_=cidx[g, :, :])
        c_offs.append(ot)
    pn_offs = []
    for g in range(N_PN):
        ot = idx_pool.tile([P, PN_GRP * K1], mybir.dt.int32, tag=f"pnoff{g}")
        nc.sync.dma_start(out=ot[:], in_=pnidx[g, :, :])
        pn_offs.append(ot)

    scores = sc_pool.tile([P, NTILES * K1], f32, tag="scores")

    # Gather center embeddings (2048 rows per core, N_C instructions).
    c_tiles = []
    for g in range(N_C):
        ct = c_pool.tile([P, C_GRP * D], DT_TAB, tag=f"ctile{g}")
        nc.gpsimd.indirect_dma_start(
            out=ct[:],
            out_offset=None,
            in_=w_in[:, :],
            in_offset=bass.IndirectOffsetOnAxis(ap=c_offs[g][:, :], axis=0),
        )
        c_tiles.append(ct)

    # Gather pos+neg embeddings and compute scores per batch tile.
    for g in range(N_PN):
        pnt = pn_pool.tile([P, PN_GRP * K1 * D], DT_TAB, tag="pnt")
        nc.gpsimd.indirect_dma_start(
            out=pnt[:],
            out_offset=None,
            in_=w_out[:, :],
            in_offset=bass.IndirectOffsetOnAxis(ap=pn_offs[g][:, :], axis=0),
        )
        if dbg is not None and g == 1:
            # raw gathered pos/neg rows for tiles 2-3 (before in-place mul)
            raw = sc_pool.tile([P, PN_GRP * K1 * D], DT_TAB, tag="pn1raw")
            nc.vector.tensor_copy(out=raw[:], in_=pnt[:])
            nc.sync.dma_start(out=dbg["pn1raw"][:, :], in_=raw[:])
        for j in range(PN_GRP):
            t = g * PN_GRP + j
            gi, lj = t // C_GRP, t % C_GRP
            ctv = c_tiles[gi][:, lj * D:(lj + 1) * D]
            pv = pnt[:, j * K1 * D:(j + 1) * K1 * D].rearrange(
                "p (k d) -> p k d", k=K1
            )
            ctb = ctv.unsqueeze(1).broadcast_to([P, K1, D])
            nc.vector.tensor_tensor(out=pv, in0=pv, in1=ctb,
                                    op=mybir.AluOpType.mult)
            nc.vector.tensor_reduce(out=scores[:, t * K1:(t + 1) * K1],
                                    in_=pv, axis=X, op=mybir.AluOpType.add)
        if dbg is not None and g == 0:
            nc.sync.dma_start(out=dbg["prod01"][:, :], in_=pnt[:])

    # loss terms: l = log(sigmoid(+/- s) + eps); sign is + for pos (k=0).
    sig = sc_pool.tile([P, NTILES * K1], f32, tag="sig")
    lg = sc_pool.tile([P, NTILES * K1], f32, tag="lg")
    s3 = scores[:].rearrange("p (t k) -> p t k", k=K1)
    g3 = sig[:].rearrange("p (t k) -> p t k", k=K1)
    nc.scalar.activation(out=g3[:, :, 0:1], in_=s3[:, :, 0:1],
                         func=AF.Sigmoid, scale=1.0)
    nc.scalar.activation(out=g3[:, :, 1:K1], in_=s3[:, :, 1:K1],
                         func=AF.Sigmoid, scale=-1.0)
    eps_t = sc_pool.tile([P, 1], f32, tag="eps")
    nc.any.memset(eps_t[:], 1e-10)
    nc.scalar.activation(out=lg[:], in_=sig[:], func=AF.Ln, bias=eps_t[:])

    colsum = sc_pool.tile([P, 1], f32, tag="colsum")
    nc.vector.tensor_reduce(out=colsum[:], in_=lg[:], axis=X,
                            op=mybir.AluOpType.add)
    ones = sc_pool.tile([P, 1], f32, tag="ones")
    nc.any.memset(ones[:], 1.0)
    acc_ps = ps_pool.tile([1, 1], f32, tag="acc")
    nc.tensor.matmul(out=acc_ps[:], lhsT=colsum[:], rhs=ones[:],
                     start=True, stop=True)
    res = sc_pool.tile([1, 1], f32, tag="res")
    nc.vector.tensor_copy(out=res[:], in_=acc_ps[:])
    nc.sync.dma_start(out=out[:, :], in_=res[:])

    if dbg is not None:
        nc.sync.dma_start(out=dbg["scores"][:, :], in_=scores[:])
        nc.sync.dma_start(out=dbg["lg"][:, :], in_=lg[:])
        nc.sync.dma_start(out=dbg["ct0"][:, :], in_=c_tiles[0][:])


def _build():
    nc = bacc.Bacc("TRN2", target_bir_lowering=False, debug=False)
    w_in = nc.dram_tensor("w_in", [V, D], DT_TAB, kind="ExternalInput")
    w_out = nc.dram_tensor("w_out", [V, D], DT_TAB, kind="ExternalInput")
    cidx = nc.dram_tensor("cidx", [N_C, P, C_GRP], mybir.dt.int32,
                          kind="ExternalInput")
    pnidx = nc.dram_tensor("pnidx", [N_PN, P, PN_GRP * K1], mybir.dt.int32,
                           kind="ExternalInput")
    out = nc.dram_tensor("out", [1, 1], mybir.dt.float32,
                         kind="ExternalOutput")
    dbg = None
    if DEBUG_DUMP:
        dbg = {
            "scores": nc.dram_tensor("scores_dbg", [P, NTILES * K1],
                                     mybir.dt.float32, kind="ExternalOutput"),
            "lg": nc.dram_tensor("lg_dbg", [P, NTILES * K1],
                                 mybir.dt.float32, kind="ExternalOutput"),
            "prod01": nc.dram_tensor("prod01_dbg", [P, PN_GRP * K1 * D],
                                     DT_TAB, kind="ExternalOutput"),
            "ct0": nc.dram_tensor("ct0_dbg", [P, C_GRP * D],
                                  DT_TAB, kind="ExternalOutput"),
            "pn1raw": nc.dram_tensor("pn1raw_dbg", [P, PN_GRP * K1 * D],
                                     DT_TAB, kind="ExternalOutput"),
        }
    with tile.TileContext(nc) as tc:
        with ExitStack() as ctx:
            _body(ctx, tc, w_in, w_out, cidx, pnidx, out, dbg)
    nc.compile()
    return nc


def _get_nc():
    global _NC
    if _NC is None:
        _NC = _build()
    return _NC


def _make_in_maps(inputs):
    center = np.asarray(inputs["center"]).astype(np.int32)
    pos = np.asarray(inputs["pos"]).astype(np.int32)
    neg = np.asarray(inputs["neg"]).astype(np.int32)
    wi = np.asarray(inputs["W_in"]).astype(NP_TAB)
    wo = np.asarray(inputs["W_out"]).astype(NP_TAB)

    in_maps = []
    for c in range(NCORES):
        sl = slice(c * BPC, (c + 1) * BPC)
        # ce[g, p, j] = center index of batch tile t=g*C_GRP+j, partition p
        ce = center[sl].reshape(N_C, C_GRP, P).transpose(0, 2, 1)
        ce = np.ascontiguousarray(ce)
        # pn[g, p, j*K1+k] = pos/neg index of tile t=g*PN_GRP+j, partition p
        pn = np.empty((NTILES, P, K1), np.int32)
        pn[:, :, 0] = pos[sl].reshape(NTILES, P)
        pn[:, :, 1:] = neg[sl].reshape(NTILES, P, NEG)
        pn = pn.reshape(N_PN, PN_GRP, P, K1).transpose(0, 2, 1, 3)
        pn = np.ascontiguousarray(pn.reshape(N_PN, P, PN_GRP * K1))
        in_maps.append({"w_in": wi, "w_out": wo, "cidx": ce, "pnidx": pn})
    return in_maps


def kernel(center, pos, neg, W_in, W_out):
    global LAST_RESULTS
    in_maps = _make_in_maps(dict(center=center, pos=pos, neg=neg,
                                 W_in=W_in, W_out=W_out))
    nc = _get_nc()
    br = run_bass_kernel_spmd(nc, in_maps, core_ids=list(range(NCORES)),
                              trace=TRACE)
    LAST_RESULTS = br
    total = sum(float(r["out"][0, 0]) for r in br.results)
    return np.float32(-total / B)

